# revision 12
# baseline (speedup 1.0000x reference)
import sys

sys.path.insert(0, "/opt/trn_rl_repo")

import numpy as np
import ml_dtypes

import concourse.bacc as bacc
import concourse.bass as bass
import concourse.mybir as mybir
import concourse.tile as tile
from concourse.bass_utils import run_bass_kernel_spmd

F32 = mybir.dt.float32
BF16 = mybir.dt.bfloat16
AF = mybir.ActivationFunctionType
ALU = mybir.AluOpType
AX = mybir.AxisListType

# Problem constants (hardcoded per harness contract).
B, C, H, W = 4, 64, 128, 128
COUT1 = 128
NT = 9          # 3x3 taps
NFF = 4         # factor*factor subpixels
NCORES = 8
HL = H // 2     # 64 coarse rows per core
NYB = 4         # y-blocks
YB = HL // NYB  # 16 rows per block
N1 = YB * 64    # per-(tap,ff) product elements per partition (16 rows x 64 c)
SPLIT_K = 6     # taps 0..SPLIT_K-1 summed on PE via identity matmuls

_cached = {}


def ap_of(t, off, dims):
    base = t[:]
    return bass.AP(base.tensor, base.offset + off, dims)


def build_nc():
    nc = bacc.Bacc("TRN2", target_bir_lowering=False, debug=False, num_devices=NCORES)

    hp2_d = nc.dram_tensor("hp2", [128, 66 * 130], BF16, kind="ExternalInput")
    ht3_d = nc.dram_tensor("ht3", [128, 66 * 192], BF16, kind="ExternalInput")
    w1a_d = nc.dram_tensor("w1a", [128, 3 * 128], BF16, kind="ExternalInput")
    w1b_d = nc.dram_tensor("w1b", [64, 3 * 128], BF16, kind="ExternalInput")
    b1_d = nc.dram_tensor("b1c", [128, 1], F32, kind="ExternalInput")
    w2t_d = nc.dram_tensor("w2t", [128, 36], BF16, kind="ExternalInput")
    eb2_d = nc.dram_tensor("eb2d", [128, 72], BF16, kind="ExternalInput")
    idq_d = nc.dram_tensor("idq", [128, 128], BF16, kind="ExternalInput")
    out_d = nc.dram_tensor("out", [64, H * 2 * W], F32, kind="ExternalOutput")

    BLOCKS = [(0, 8), (8, 16), (24, 16), (40, 16), (56, 8)]

    with tile.TileContext(nc) as tc:
        with (
            tc.tile_pool(name="const", bufs=1) as cpool,
            tc.tile_pool(name="ring", bufs=2) as ring,
            tc.tile_pool(name="mpool", bufs=2) as mpool,
            tc.tile_pool(name="spool", bufs=2) as spool,
            tc.tile_pool(name="prodp", bufs=3) as prodp,
            tc.tile_pool(name="dpool", bufs=3) as dpool,
            tc.tile_pool(name="accp", bufs=2) as accp,
            tc.tile_pool(name="orow", bufs=3) as opool,
            tc.tile_pool(name="ps1", bufs=2, space=bass.MemorySpace.PSUM) as pp1,
            tc.tile_pool(name="psE", bufs=2, space=bass.MemorySpace.PSUM) as ppE,
            tc.tile_pool(name="psA", bufs=3, space=bass.MemorySpace.PSUM) as ppA,
            tc.tile_pool(name="pso", bufs=1, space=bass.MemorySpace.PSUM) as ppo,
        ):
            # ---- first block's inputs first, then constants (the sync queue
            # issues DMAs serially at ~600ns each; the first conv only needs
            # w1a/w1b + hp2b) ----
            r0_0, R_0 = BLOCKS[0]
            hp2b0 = ring.tile([128, 18 * 130], BF16, tag="hp2b")
            ht3b0 = ring.tile([128, 18 * 192], BF16, tag="ht3b")
            nc.sync.dma_start(hp2b0[:, 0:(R_0 + 2) * 130],
                              hp2_d[:, r0_0 * 130:(r0_0 + R_0 + 2) * 130])
            w1a = cpool.tile([128, 3 * 128], BF16)
            w1b = cpool.tile([64, 3 * 128], BF16)
            b1 = cpool.tile([128, 1], F32)
            w2t = cpool.tile([128, 36], BF16)
            eb2 = cpool.tile([128, 72], BF16)
            idq = cpool.tile([128, 128], BF16)
            nc.sync.dma_start(w1a[:], w1a_d[:])
            nc.sync.dma_start(w1b[:], w1b_d[:])
            nc.sync.dma_start(b1[:], b1_d[:])
            nc.sync.dma_start(ht3b0[:, 0:(R_0 + 2) * 192],
                              ht3_d[:, r0_0 * 192:(r0_0 + R_0 + 2) * 192])
            nc.sync.dma_start(w2t[:], w2t_d[:])
            nc.sync.dma_start(eb2[:], eb2_d[:])
            nc.sync.dma_start(idq[:], idq_d[:])

            def emit_front(r0, R, tiles=None):
                """DMA + conv1 -> relu -> conv2(rows) -> exp -> softmax."""
                if tiles is None:
                    hp2b = ring.tile([128, 18 * 130], BF16, tag="hp2b")
                    ht3b = ring.tile([128, 18 * 192], BF16, tag="ht3b")
                    nc.sync.dma_start(hp2b[:, 0:(R + 2) * 130],
                                      hp2_d[:, r0 * 130:(r0 + R + 2) * 130])
                    nc.sync.dma_start(ht3b[:, 0:(R + 2) * 192],
                                      ht3_d[:, r0 * 192:(r0 + R + 2) * 192])
                else:
                    hp2b, ht3b = tiles

                m = mpool.tile([128, 2048], BF16, tag="m")
                eT2 = spool.tile([128, 16 * 72], BF16, tag="eT2")

                def conv1(ic):
                    ps1 = pp1.tile([128, 512], F32)
                    for dy in range(3):
                        rhs = ap_of(hp2b, (4 * ic + dy) * 130,
                                    [[18 * 130, 128], [130, 4], [1, 128]])
                        nc.tensor.matmul(ps1[:], w1a[:, dy * 128:(dy + 1) * 128],
                                         rhs, start=(dy == 0), stop=False)
                    for dy in range(3):
                        rhs = ap_of(hp2b, (4 * ic + dy) * 130 + 2,
                                    [[18 * 130, 64], [130, 4], [1, 128]])
                        nc.tensor.matmul(ps1[:], w1b[:, dy * 128:(dy + 1) * 128],
                                         rhs, start=False, stop=(dy == 2))
                    nc.scalar.activation(m[:, ic * 512:(ic + 1) * 512], ps1[:],
                                         AF.Relu, bias=b1[:], scale=1.0)

                def conv2(ic):
                    psE = ppE.tile([128, 160], F32)
                    for rl in range(4):
                        r = 4 * ic + rl
                        nc.tensor.matmul(psE[:, rl * 40:rl * 40 + 36],
                                         m[:, r * 128:(r + 1) * 128], w2t[:])
                    # exp with free-dup x2: eT2[x, (4r, 36, 2)]
                    e_out = ap_of(eT2, ic * 4 * 72,
                                  [[16 * 72, 128], [72, 4], [2, 36], [1, 2]])
                    e_in = ap_of(psE, 0, [[160, 128], [40, 4], [1, 36], [0, 2]])
                    nc.scalar.activation(e_out, e_in, AF.Exp, scale=1.0)

                # chunk-level software pipeline: conv1(ic+1) before conv2(ic)
                nch = R // 4
                conv1(0)
                for ic in range(1, nch):
                    conv1(ic)
                    conv2(ic - 1)
                conv2(nch - 1)

                # ---- softmax pieces (transposed layout, x on partitions) ----
                q2 = spool.tile([128, 16 * 72], BF16, tag="q2")
                in_e = ap_of(eT2, 0, [[16 * 72, 128], [72, R], [1, 72]])
                in_b = ap_of(eb2, 0, [[72, 128], [0, R], [1, 72]])
                q_out = ap_of(q2, 0, [[16 * 72, 128], [72, R], [1, 72]])
                nc.vector.tensor_tensor(q_out, in_e, in_b, ALU.mult)

                zt = spool.tile([128, 64], F32, tag="zt")
                rz = spool.tile([128, 64], F32, tag="rz")
                rzd = spool.tile([128, 128], BF16, tag="rzd")
                z_in = ap_of(q2, 0, [[16 * 72, 128], [72, R], [18, 4], [2, 9]])
                z_out = ap_of(zt, 0, [[64, 128], [1, R * 4]])
                nc.vector.tensor_reduce(z_out, z_in, AX.X, ALU.add)
                nc.vector.reciprocal(ap_of(rz, 0, [[64, 128], [1, R * 4]]),
                                     ap_of(zt, 0, [[64, 128], [1, R * 4]]))
                rzd_out = ap_of(rzd, 0, [[128, 128], [2, R * 4], [1, 2]])
                rzd_in = ap_of(rz, 0, [[64, 128], [1, R * 4], [0, 2]])
                nc.scalar.copy(rzd_out, rzd_in)

                # nm[x, (ff, r, t, 2)] = q2 * rz  (bf16, dup x2 for 2x mode)
                nm = spool.tile([128, NFF * 16 * 18], BF16, tag="nm")
                for ff in range(NFF):
                    o = ap_of(nm, ff * R * 18,
                              [[NFF * 16 * 18, 128], [18, R], [1, 18]])
                    i0 = ap_of(q2, ff * 18, [[16 * 72, 128], [72, R], [1, 18]])
                    i1 = ap_of(rzd, ff * 2, [[128, 128], [8, R], [0, 9], [1, 2]])
                    nc.vector.tensor_tensor(o, i0, i1, ALU.mult)
                return ht3b, nm

            def emit_products(R, ht3b, nm):
                """DVE tap products + PE identity-matmul tap-sum -> acc."""
                n = R * 64
                acc = accp.tile([128, NFF * 1024], BF16, tag="acc")

                def prod_ap(prod, t):
                    return ap_of(prod, t * n,
                                 [[NT * 1024, 128], [64, R], [2, 32], [1, 2]])

                def tap_ins(ff, t):
                    dy, dx = t // 3, t % 3
                    i0 = ap_of(ht3b, dy * 192 + dx * 64,
                               [[18 * 192, 128], [192, R], [2, 32], [1, 2]])
                    i1 = ap_of(nm, ff * R * 18 + t * 2,
                               [[NFF * 16 * 18, 128], [18, R], [0, 32], [1, 2]])
                    return i0, i1

                def finish_ff(ff, prod, tD):
                    # DVE: D = p6 + (p7 + p8)
                    nc.vector.tensor_add(
                        ap_of(tD, n, [[2 * 1024, 128], [1, n]]),
                        ap_of(prod, 6 * n, [[NT * 1024, 128], [1, n]]),
                        ap_of(tD, 0, [[2 * 1024, 128], [1, n]]))
                    for half in range(n // 512):
                        psacc = ppA.tile([128, 512], F32)
                        for t in range(SPLIT_K):
                            rhs = ap_of(prod, t * n + half * 512,
                                        [[NT * 1024, 128], [1, 512]])
                            nc.tensor.matmul(psacc[:], idq[:], rhs,
                                             start=(t == 0), stop=False)
                        rhs = ap_of(tD, n + half * 512, [[2 * 1024, 128], [1, 512]])
                        nc.tensor.matmul(psacc[:], idq[:], rhs,
                                         start=False, stop=True)
                        # acc layout: [x, (r, fx, fy, c)]; ff = fy*2 + fx
                        fy, fx = ff // 2, ff % 2
                        a_out = ap_of(acc, (half * 8) * 256 + fx * 128 + fy * 64,
                                      [[NFF * 1024, 128], [256, 8], [1, 64]])
                        a_in = ap_of(psacc, 0, [[512, 128], [64, 8], [1, 64]])
                        nc.scalar.copy(a_out, a_in)

                pending = None
                for ff in range(NFF):
                    prod = prodp.tile([128, NT * 1024], BF16, tag="prod")
                    tD = dpool.tile([128, 2 * 1024], BF16, tag="tD")
                    for t in range(NT):
                        i0, i1 = tap_ins(ff, t)
                        nc.vector.tensor_tensor(prod_ap(prod, t), i0, i1, ALU.mult)
                    nc.vector.tensor_add(
                        ap_of(tD, 0, [[2 * 1024, 128], [1, n]]),
                        ap_of(prod, 7 * n, [[NT * 1024, 128], [1, n]]),
                        ap_of(prod, 8 * n, [[NT * 1024, 128], [1, n]]))
                    if pending is not None:
                        finish_ff(*pending)
                    pending = (ff, prod, tD)
                finish_ff(*pending)
                return acc

            def emit_out(r0, R, acc):
                """Pixel-shuffle transposes + copies + output DMA."""
                for yg in range(R // 4):
                    psoB = ppo.tile([128, 1024], BF16)
                    for yl_loc in range(4):
                        yl = yg * 4 + yl_loc
                        for fx in range(2):
                            t_in = ap_of(acc, yl * 256 + fx * 128,
                                         [[NFF * 1024, 128], [1, 128]])
                            nc.tensor.transpose(
                                psoB[:, yl_loc * 256 + fx * 128:
                                     yl_loc * 256 + (fx + 1) * 128],
                                t_in, idq[:])
                    orow4 = opool.tile([128, 1024], F32)
                    for fx in range(2):
                        o_ap = ap_of(orow4, fx, [[1024, 128], [256, 4], [2, 128]])
                        i_ap = ap_of(psoB, fx * 128,
                                     [[1024, 128], [256, 4], [1, 128]])
                        nc.scalar.copy(o_ap, i_ap)
                    for fy in range(2):
                        od = ap_of(out_d, (2 * r0 + 8 * yg + fy) * 256,
                                   [[2 * H * W, 64], [512, 4], [1, 256]])
                        nc.sync.dma_start(od, orow4[fy * 64:(fy + 1) * 64, :])

            # block-level software pipeline: defer each block's output until
            # after the next block's conv/softmax is emitted, so the PE queue
            # is never blocked on DVE products when a new conv could run.
            prev_out = None
            first_tiles = (hp2b0, ht3b0)
            for bi, (r0, R) in enumerate(BLOCKS):
                ht3b, nm = emit_front(r0, R, first_tiles if bi == 0 else None)
                if prev_out is not None:
                    emit_out(*prev_out)
                acc = emit_products(R, ht3b, nm)
                prev_out = (r0, R, acc)
            emit_out(*prev_out)

    nc.compile()
    return nc


def prep_shared(W1, b1, W2, b2):
    W1 = np.asarray(W1, np.float32)
    b1 = np.asarray(b1, np.float32)
    W2 = np.asarray(W2, np.float32).reshape(36, 128)
    b2 = np.asarray(b2, np.float32)

    w1a = np.zeros((128, 3 * 128), np.float32)
    w1b = np.zeros((64, 3 * 128), np.float32)
    for dy in range(3):
        w1a[0:64, dy * 128:(dy + 1) * 128] = W1[:, :, dy, 0].T
        w1a[64:128, dy * 128:(dy + 1) * 128] = W1[:, :, dy, 1].T
        w1b[:, dy * 128:(dy + 1) * 128] = W1[:, :, dy, 2].T

    # w2t columns k = ff*9 + t  ->  original channel t*4 + ff, 0.25 folded in
    o_of_mp = np.array([t * 4 + ff for ff in range(4) for t in range(9)])
    w2t = np.ascontiguousarray((0.25 * W2[o_of_mp, :]).T)
    eb2 = np.exp(0.25 * b2[o_of_mp]).astype(np.float32)        # [36]
    eb2d = np.broadcast_to(np.repeat(eb2, 2)[None, :], (128, 72))

    bf = ml_dtypes.bfloat16
    return {
        "w1a": w1a.astype(bf), "w1b": w1b.astype(bf),
        "b1c": b1.reshape(128, 1).astype(np.float32),
        "w2t": w2t.astype(bf),
        "eb2d": np.ascontiguousarray(eb2d).astype(bf),
        "idq": np.eye(128, dtype=bf),
    }


def kernel(h, W1, b1, W2, b2, _trace=False):
    h = np.asarray(h, np.float32)
    shared = prep_shared(W1, b1, W2, b2)

    hp = np.pad(h, ((0, 0), (0, 0), (1, 1), (1, 1)))  # [B, C, 130, 130]
    bf = ml_dtypes.bfloat16
    in_maps = []
    for core in range(NCORES):
        b, half = core // 2, core % 2
        y0 = half * HL
        win = hp[b, :, y0:y0 + 66, :]  # [64, 66, 130]
        hp2 = np.zeros((128, 66, 130), np.float32)
        hp2[0:64] = win
        hp2[64:128, :, 0:129] = win[:, :, 1:130]
        # ht3[x, y, dx, c] = 8*win[c, y, x+dx]
        w8 = 8.0 * win
        ht3 = np.stack([w8[:, :, dx:dx + 128] for dx in range(3)],
                       axis=0).transpose(3, 2, 0, 1)  # [128, 66, 3, 64]
        m = dict(shared)
        m["hp2"] = np.ascontiguousarray(hp2.reshape(128, -1)).astype(bf)
        m["ht3"] = np.ascontiguousarray(ht3.reshape(128, -1)).astype(bf)
        in_maps.append(m)

    if "nc" not in _cached:
        _cached["nc"] = build_nc()
    res = run_bass_kernel_spmd(_cached["nc"], in_maps, core_ids=list(range(NCORES)),
                               trace=_trace)

    out = np.zeros((B, C, 2 * H, 2 * W), np.float32)
    for core in range(NCORES):
        b, half = core // 2, core % 2
        out[b, :, half * 128:(half + 1) * 128, :] = \
            res.results[core]["out"].reshape(64, H, 2 * W)
    if _trace:
        return out, res
    return out


# revision 13
# speedup vs baseline: 1.1071x; 1.1071x over previous
import sys

sys.path.insert(0, "/opt/trn_rl_repo")

import numpy as np
import ml_dtypes

import concourse.bacc as bacc
import concourse.bass as bass
import concourse.mybir as mybir
import concourse.tile as tile
from concourse.bass_utils import run_bass_kernel_spmd

F32 = mybir.dt.float32
BF16 = mybir.dt.bfloat16
AF = mybir.ActivationFunctionType
ALU = mybir.AluOpType
AX = mybir.AxisListType

# Problem constants (hardcoded per harness contract).
B, C, H, W = 4, 64, 128, 128
COUT1 = 128
NT = 9          # 3x3 taps
NFF = 4         # factor*factor subpixels
NCORES = 8
HL = H // 2     # 64 coarse rows per core
NYB = 4         # y-blocks
YB = HL // NYB  # 16 rows per block
N1 = YB * 64    # per-(tap,ff) product elements per partition (16 rows x 64 c)
SPLIT_K = 6     # taps 0..SPLIT_K-1 summed on PE via identity matmuls

_cached = {}


def ap_of(t, off, dims):
    base = t[:]
    return bass.AP(base.tensor, base.offset + off, dims)


def build_nc():
    nc = bacc.Bacc("TRN2", target_bir_lowering=False, debug=False, num_devices=NCORES)

    hp2_d = nc.dram_tensor("hp2", [128, 66 * 130], BF16, kind="ExternalInput")
    ht3_d = nc.dram_tensor("ht3", [128, 66 * 192], BF16, kind="ExternalInput")
    w1a_d = nc.dram_tensor("w1a", [128, 3 * 128], BF16, kind="ExternalInput")
    w1b_d = nc.dram_tensor("w1b", [64, 3 * 128], BF16, kind="ExternalInput")
    b1_d = nc.dram_tensor("b1c", [128, 1], F32, kind="ExternalInput")
    w2t_d = nc.dram_tensor("w2t", [128, 36], BF16, kind="ExternalInput")
    eb2_d = nc.dram_tensor("eb2d", [128, 72], BF16, kind="ExternalInput")
    idq_d = nc.dram_tensor("idq", [128, 128], BF16, kind="ExternalInput")
    out_d = nc.dram_tensor("out", [64, H * 2 * W], F32, kind="ExternalOutput")

    BLOCKS = [(0, 16), (16, 16), (32, 16), (48, 16)]

    with tile.TileContext(nc) as tc:
        with (
            tc.tile_pool(name="const", bufs=1) as cpool,
            tc.tile_pool(name="ring", bufs=2) as ring,
            tc.tile_pool(name="mpool", bufs=2) as mpool,
            tc.tile_pool(name="spool", bufs=2) as spool,
            tc.tile_pool(name="prodp", bufs=3) as prodp,
            tc.tile_pool(name="dpool", bufs=3) as dpool,
            tc.tile_pool(name="accp", bufs=2) as accp,
            tc.tile_pool(name="orow", bufs=3) as opool,
            tc.tile_pool(name="ps1", bufs=2, space=bass.MemorySpace.PSUM) as pp1,
            tc.tile_pool(name="psE", bufs=2, space=bass.MemorySpace.PSUM) as ppE,
            tc.tile_pool(name="psA", bufs=3, space=bass.MemorySpace.PSUM) as ppA,
            tc.tile_pool(name="pso", bufs=1, space=bass.MemorySpace.PSUM) as ppo,
        ):
            # ---- first block's inputs first, then constants (the sync queue
            # issues DMAs serially at ~600ns each; the first conv only needs
            # w1a/w1b + hp2b) ----
            r0_0, R_0 = BLOCKS[0]
            hp2b0 = ring.tile([128, 18 * 130], BF16, tag="hp2b")
            ht3b0 = ring.tile([128, 18 * 192], BF16, tag="ht3b")
            nc.sync.dma_start(hp2b0[:, 0:(R_0 + 2) * 130],
                              hp2_d[:, r0_0 * 130:(r0_0 + R_0 + 2) * 130])
            w1a = cpool.tile([128, 3 * 128], BF16)
            w1b = cpool.tile([64, 3 * 128], BF16)
            b1 = cpool.tile([128, 1], F32)
            w2t = cpool.tile([128, 36], BF16)
            eb2 = cpool.tile([128, 72], BF16)
            idq = cpool.tile([128, 128], BF16)
            nc.sync.dma_start(w1a[:], w1a_d[:])
            nc.sync.dma_start(w1b[:], w1b_d[:])
            nc.sync.dma_start(b1[:], b1_d[:])
            nc.sync.dma_start(ht3b0[:, 0:(R_0 + 2) * 192],
                              ht3_d[:, r0_0 * 192:(r0_0 + R_0 + 2) * 192])
            nc.sync.dma_start(w2t[:], w2t_d[:])
            nc.sync.dma_start(eb2[:], eb2_d[:])
            nc.sync.dma_start(idq[:], idq_d[:])

            def emit_front(r0, R, tiles=None):
                """DMA + conv1 -> relu -> conv2(rows) -> exp -> softmax."""
                if tiles is None:
                    hp2b = ring.tile([128, 18 * 130], BF16, tag="hp2b")
                    ht3b = ring.tile([128, 18 * 192], BF16, tag="ht3b")
                    nc.sync.dma_start(hp2b[:, 0:(R + 2) * 130],
                                      hp2_d[:, r0 * 130:(r0 + R + 2) * 130])
                    nc.sync.dma_start(ht3b[:, 0:(R + 2) * 192],
                                      ht3_d[:, r0 * 192:(r0 + R + 2) * 192])
                else:
                    hp2b, ht3b = tiles

                m = mpool.tile([128, 2048], BF16, tag="m")
                eT2 = spool.tile([128, 16 * 72], BF16, tag="eT2")

                def conv1(ic):
                    ps1 = pp1.tile([128, 512], F32)
                    for dy in range(3):
                        rhs = ap_of(hp2b, (4 * ic + dy) * 130,
                                    [[18 * 130, 128], [130, 4], [1, 128]])
                        nc.tensor.matmul(ps1[:], w1a[:, dy * 128:(dy + 1) * 128],
                                         rhs, start=(dy == 0), stop=False)
                    for dy in range(3):
                        rhs = ap_of(hp2b, (4 * ic + dy) * 130 + 2,
                                    [[18 * 130, 64], [130, 4], [1, 128]])
                        nc.tensor.matmul(ps1[:], w1b[:, dy * 128:(dy + 1) * 128],
                                         rhs, start=False, stop=(dy == 2))
                    nc.scalar.activation(m[:, ic * 512:(ic + 1) * 512], ps1[:],
                                         AF.Relu, bias=b1[:], scale=1.0)

                def conv2(ic):
                    psE = ppE.tile([128, 160], F32)
                    for rl in range(4):
                        r = 4 * ic + rl
                        nc.tensor.matmul(psE[:, rl * 40:rl * 40 + 36],
                                         m[:, r * 128:(r + 1) * 128], w2t[:])
                    # exp with free-dup x2: eT2[x, (4r, 36, 2)]
                    e_out = ap_of(eT2, ic * 4 * 72,
                                  [[16 * 72, 128], [72, 4], [2, 36], [1, 2]])
                    e_in = ap_of(psE, 0, [[160, 128], [40, 4], [1, 36], [0, 2]])
                    nc.scalar.activation(e_out, e_in, AF.Exp, scale=1.0)

                # chunk-level software pipeline: conv1(ic+1) before conv2(ic)
                nch = R // 4
                conv1(0)
                for ic in range(1, nch):
                    conv1(ic)
                    conv2(ic - 1)
                conv2(nch - 1)

                # ---- softmax pieces (transposed layout, x on partitions) ----
                q2 = spool.tile([128, 16 * 72], BF16, tag="q2")
                in_e = ap_of(eT2, 0, [[16 * 72, 128], [72, R], [1, 72]])
                in_b = ap_of(eb2, 0, [[72, 128], [0, R], [1, 72]])
                q_out = ap_of(q2, 0, [[16 * 72, 128], [72, R], [1, 72]])
                nc.vector.tensor_tensor(q_out, in_e, in_b, ALU.mult)

                zt = spool.tile([128, 64], F32, tag="zt")
                rz = spool.tile([128, 64], F32, tag="rz")
                rzd = spool.tile([128, 128], BF16, tag="rzd")
                z_in = ap_of(q2, 0, [[16 * 72, 128], [72, R], [18, 4], [2, 9]])
                z_out = ap_of(zt, 0, [[64, 128], [1, R * 4]])
                nc.vector.tensor_reduce(z_out, z_in, AX.X, ALU.add)
                nc.vector.reciprocal(ap_of(rz, 0, [[64, 128], [1, R * 4]]),
                                     ap_of(zt, 0, [[64, 128], [1, R * 4]]))
                rzd_out = ap_of(rzd, 0, [[128, 128], [2, R * 4], [1, 2]])
                rzd_in = ap_of(rz, 0, [[64, 128], [1, R * 4], [0, 2]])
                nc.scalar.copy(rzd_out, rzd_in)

                # nm[x, (ff, r, t, 2)] = q2 * rz  (bf16, dup x2 for 2x mode)
                nm = spool.tile([128, NFF * 16 * 18], BF16, tag="nm")
                for ff in range(NFF):
                    o = ap_of(nm, ff * R * 18,
                              [[NFF * 16 * 18, 128], [18, R], [1, 18]])
                    i0 = ap_of(q2, ff * 18, [[16 * 72, 128], [72, R], [1, 18]])
                    i1 = ap_of(rzd, ff * 2, [[128, 128], [8, R], [0, 9], [1, 2]])
                    nc.vector.tensor_tensor(o, i0, i1, ALU.mult)
                return ht3b, nm

            def emit_products(R, ht3b, nm):
                """DVE tap products + PE identity-matmul tap-sum -> acc."""
                n = R * 64
                acc = accp.tile([128, NFF * 1024], BF16, tag="acc")

                def prod_ap(prod, t):
                    return ap_of(prod, t * n,
                                 [[NT * 1024, 128], [64, R], [2, 32], [1, 2]])

                def tap_ins(ff, t):
                    dy, dx = t // 3, t % 3
                    i0 = ap_of(ht3b, dy * 192 + dx * 64,
                               [[18 * 192, 128], [192, R], [2, 32], [1, 2]])
                    i1 = ap_of(nm, ff * R * 18 + t * 2,
                               [[NFF * 16 * 18, 128], [18, R], [0, 32], [1, 2]])
                    return i0, i1

                def finish_ff(ff, prod, tD):
                    # DVE: D = p6 + (p7 + p8)
                    nc.vector.tensor_add(
                        ap_of(tD, n, [[2 * 1024, 128], [1, n]]),
                        ap_of(prod, 6 * n, [[NT * 1024, 128], [1, n]]),
                        ap_of(tD, 0, [[2 * 1024, 128], [1, n]]))
                    for half in range(n // 512):
                        psacc = ppA.tile([128, 512], F32)
                        for t in range(SPLIT_K):
                            rhs = ap_of(prod, t * n + half * 512,
                                        [[NT * 1024, 128], [1, 512]])
                            nc.tensor.matmul(psacc[:], idq[:], rhs,
                                             start=(t == 0), stop=False)
                        rhs = ap_of(tD, n + half * 512, [[2 * 1024, 128], [1, 512]])
                        nc.tensor.matmul(psacc[:], idq[:], rhs,
                                         start=False, stop=True)
                        # acc layout: [x, (r, fx, fy, c)]; ff = fy*2 + fx
                        fy, fx = ff // 2, ff % 2
                        a_out = ap_of(acc, (half * 8) * 256 + fx * 128 + fy * 64,
                                      [[NFF * 1024, 128], [256, 8], [1, 64]])
                        a_in = ap_of(psacc, 0, [[512, 128], [64, 8], [1, 64]])
                        nc.scalar.copy(a_out, a_in)

                pending = None
                for ff in range(NFF):
                    prod = prodp.tile([128, NT * 1024], BF16, tag="prod")
                    tD = dpool.tile([128, 2 * 1024], BF16, tag="tD")
                    for t in range(NT):
                        i0, i1 = tap_ins(ff, t)
                        nc.vector.tensor_tensor(prod_ap(prod, t), i0, i1, ALU.mult)
                    nc.vector.tensor_add(
                        ap_of(tD, 0, [[2 * 1024, 128], [1, n]]),
                        ap_of(prod, 7 * n, [[NT * 1024, 128], [1, n]]),
                        ap_of(prod, 8 * n, [[NT * 1024, 128], [1, n]]))
                    if pending is not None:
                        finish_ff(*pending)
                    pending = (ff, prod, tD)
                finish_ff(*pending)
                return acc

            def emit_out(r0, R, acc):
                """Pixel-shuffle transposes + copies + output DMA."""
                for yg in range(R // 4):
                    psoB = ppo.tile([128, 1024], BF16)
                    for yl_loc in range(4):
                        yl = yg * 4 + yl_loc
                        for fx in range(2):
                            t_in = ap_of(acc, yl * 256 + fx * 128,
                                         [[NFF * 1024, 128], [1, 128]])
                            nc.tensor.transpose(
                                psoB[:, yl_loc * 256 + fx * 128:
                                     yl_loc * 256 + (fx + 1) * 128],
                                t_in, idq[:])
                    orow4 = opool.tile([128, 1024], F32)
                    for fx in range(2):
                        o_ap = ap_of(orow4, fx, [[1024, 128], [256, 4], [2, 128]])
                        i_ap = ap_of(psoB, fx * 128,
                                     [[1024, 128], [256, 4], [1, 128]])
                        nc.scalar.copy(o_ap, i_ap)
                    for fy in range(2):
                        od = ap_of(out_d, (2 * r0 + 8 * yg + fy) * 256,
                                   [[2 * H * W, 64], [512, 4], [1, 256]])
                        nc.sync.dma_start(od, orow4[fy * 64:(fy + 1) * 64, :])

            # block-level software pipeline: defer each block's output until
            # after the next block's conv/softmax is emitted, so the PE queue
            # is never blocked on DVE products when a new conv could run.
            prev_out = None
            first_tiles = (hp2b0, ht3b0)
            for bi, (r0, R) in enumerate(BLOCKS):
                ht3b, nm = emit_front(r0, R, first_tiles if bi == 0 else None)
                if prev_out is not None:
                    emit_out(*prev_out)
                acc = emit_products(R, ht3b, nm)
                prev_out = (r0, R, acc)
            emit_out(*prev_out)

    nc.compile()
    return nc


def prep_shared(W1, b1, W2, b2):
    W1 = np.asarray(W1, np.float32)
    b1 = np.asarray(b1, np.float32)
    W2 = np.asarray(W2, np.float32).reshape(36, 128)
    b2 = np.asarray(b2, np.float32)

    w1a = np.zeros((128, 3 * 128), np.float32)
    w1b = np.zeros((64, 3 * 128), np.float32)
    for dy in range(3):
        w1a[0:64, dy * 128:(dy + 1) * 128] = W1[:, :, dy, 0].T
        w1a[64:128, dy * 128:(dy + 1) * 128] = W1[:, :, dy, 1].T
        w1b[:, dy * 128:(dy + 1) * 128] = W1[:, :, dy, 2].T

    # w2t columns k = ff*9 + t  ->  original channel t*4 + ff, 0.25 folded in
    o_of_mp = np.array([t * 4 + ff for ff in range(4) for t in range(9)])
    w2t = np.ascontiguousarray((0.25 * W2[o_of_mp, :]).T)
    eb2 = np.exp(0.25 * b2[o_of_mp]).astype(np.float32)        # [36]
    eb2d = np.broadcast_to(np.repeat(eb2, 2)[None, :], (128, 72))

    bf = ml_dtypes.bfloat16
    return {
        "w1a": w1a.astype(bf), "w1b": w1b.astype(bf),
        "b1c": b1.reshape(128, 1).astype(np.float32),
        "w2t": w2t.astype(bf),
        "eb2d": np.ascontiguousarray(eb2d).astype(bf),
        "idq": np.eye(128, dtype=bf),
    }


def kernel(h, W1, b1, W2, b2, _trace=False):
    h = np.asarray(h, np.float32)
    shared = prep_shared(W1, b1, W2, b2)

    hp = np.pad(h, ((0, 0), (0, 0), (1, 1), (1, 1)))  # [B, C, 130, 130]
    bf = ml_dtypes.bfloat16
    in_maps = []
    for core in range(NCORES):
        b, half = core // 2, core % 2
        y0 = half * HL
        win = hp[b, :, y0:y0 + 66, :]  # [64, 66, 130]
        hp2 = np.zeros((128, 66, 130), np.float32)
        hp2[0:64] = win
        hp2[64:128, :, 0:129] = win[:, :, 1:130]
        # ht3[x, y, dx, c] = 8*win[c, y, x+dx]
        w8 = 8.0 * win
        ht3 = np.stack([w8[:, :, dx:dx + 128] for dx in range(3)],
                       axis=0).transpose(3, 2, 0, 1)  # [128, 66, 3, 64]
        m = dict(shared)
        m["hp2"] = np.ascontiguousarray(hp2.reshape(128, -1)).astype(bf)
        m["ht3"] = np.ascontiguousarray(ht3.reshape(128, -1)).astype(bf)
        in_maps.append(m)

    if "nc" not in _cached:
        _cached["nc"] = build_nc()
    res = run_bass_kernel_spmd(_cached["nc"], in_maps, core_ids=list(range(NCORES)),
                               trace=_trace)

    out = np.zeros((B, C, 2 * H, 2 * W), np.float32)
    for core in range(NCORES):
        b, half = core // 2, core % 2
        out[b, :, half * 128:(half + 1) * 128, :] = \
            res.results[core]["out"].reshape(64, H, 2 * W)
    if _trace:
        return out, res
    return out


# revision 16
# speedup vs baseline: 1.1256x; 1.0167x over previous
import sys

sys.path.insert(0, "/opt/trn_rl_repo")

import numpy as np
import ml_dtypes

import concourse.bacc as bacc
import concourse.bass as bass
import concourse.mybir as mybir
import concourse.tile as tile
from concourse.bass_utils import run_bass_kernel_spmd

F32 = mybir.dt.float32
BF16 = mybir.dt.bfloat16
AF = mybir.ActivationFunctionType
ALU = mybir.AluOpType
AX = mybir.AxisListType

# Problem constants (hardcoded per harness contract).
B, C, H, W = 4, 64, 128, 128
COUT1 = 128
NT = 9          # 3x3 taps
NFF = 4         # factor*factor subpixels
NCORES = 8
HL = H // 2     # 64 coarse rows per core
NYB = 4         # y-blocks
YB = HL // NYB  # 16 rows per block
N1 = YB * 64    # per-(tap,ff) product elements per partition (16 rows x 64 c)
SPLIT_K = 6     # taps 0..SPLIT_K-1 summed on PE via identity matmuls

_cached = {}


def ap_of(t, off, dims):
    base = t[:]
    return bass.AP(base.tensor, base.offset + off, dims)


def build_nc():
    nc = bacc.Bacc("TRN2", target_bir_lowering=False, debug=False, num_devices=NCORES)

    hp2_d = nc.dram_tensor("hp2", [128, 66 * 130], BF16, kind="ExternalInput")
    ht3_d = nc.dram_tensor("ht3", [128, 66 * 192], BF16, kind="ExternalInput")
    w1a_d = nc.dram_tensor("w1a", [128, 3 * 128], BF16, kind="ExternalInput")
    w1b_d = nc.dram_tensor("w1b", [64, 3 * 128], BF16, kind="ExternalInput")
    b1_d = nc.dram_tensor("b1c", [128, 1], F32, kind="ExternalInput")
    w2t_d = nc.dram_tensor("w2t", [128, 36], BF16, kind="ExternalInput")
    eb2_d = nc.dram_tensor("eb2d", [128, 72], BF16, kind="ExternalInput")
    idq_d = nc.dram_tensor("idq", [128, 128], BF16, kind="ExternalInput")
    out_d = nc.dram_tensor("out", [64, H * 2 * W], F32, kind="ExternalOutput")

    BLOCKS = [(0, 16), (16, 16), (32, 16), (48, 16)]

    with tile.TileContext(nc) as tc:
        with (
            tc.tile_pool(name="const", bufs=1) as cpool,
            tc.tile_pool(name="ring", bufs=2) as ring,
            tc.tile_pool(name="mpool", bufs=2) as mpool,
            tc.tile_pool(name="spool", bufs=2) as spool,
            tc.tile_pool(name="prodp", bufs=3) as prodp,
            tc.tile_pool(name="dpool", bufs=3) as dpool,
            tc.tile_pool(name="accp", bufs=2) as accp,
            tc.tile_pool(name="orow", bufs=3) as opool,
            tc.tile_pool(name="ps1", bufs=2, space=bass.MemorySpace.PSUM) as pp1,
            tc.tile_pool(name="psE", bufs=2, space=bass.MemorySpace.PSUM) as ppE,
            tc.tile_pool(name="psA", bufs=3, space=bass.MemorySpace.PSUM) as ppA,
            tc.tile_pool(name="pso", bufs=1, space=bass.MemorySpace.PSUM) as ppo,
        ):
            # ---- first block's inputs first, then constants (the sync queue
            # issues DMAs serially at ~600ns each; the first conv only needs
            # w1a/w1b + hp2b) ----
            r0_0, R_0 = BLOCKS[0]
            hp2b0 = ring.tile([128, 18 * 130], BF16, tag="hp2b")
            ht3b0 = ring.tile([128, 18 * 192], BF16, tag="ht3b")
            nc.sync.dma_start(hp2b0[:, 0:(R_0 + 2) * 130],
                              hp2_d[:, r0_0 * 130:(r0_0 + R_0 + 2) * 130])
            w1a = cpool.tile([128, 3 * 128], BF16)
            w1b = cpool.tile([64, 3 * 128], BF16)
            b1 = cpool.tile([128, 1], F32)
            w2t = cpool.tile([128, 36], BF16)
            eb2 = cpool.tile([128, 72], BF16)
            idq = cpool.tile([128, 128], BF16)
            nc.sync.dma_start(w1a[:], w1a_d[:])
            nc.sync.dma_start(w1b[:], w1b_d[:])
            nc.sync.dma_start(b1[:], b1_d[:])
            nc.sync.dma_start(ht3b0[:, 0:(R_0 + 2) * 192],
                              ht3_d[:, r0_0 * 192:(r0_0 + R_0 + 2) * 192])
            nc.sync.dma_start(w2t[:], w2t_d[:])
            nc.sync.dma_start(eb2[:], eb2_d[:])
            nc.sync.dma_start(idq[:], idq_d[:])

            def emit_front(r0, R, tiles=None):
                """DMA + conv1 -> relu -> conv2(rows) -> exp -> softmax."""
                if tiles is None:
                    hp2b = ring.tile([128, 18 * 130], BF16, tag="hp2b")
                    ht3b = ring.tile([128, 18 * 192], BF16, tag="ht3b")
                    nc.sync.dma_start(hp2b[:, 0:(R + 2) * 130],
                                      hp2_d[:, r0 * 130:(r0 + R + 2) * 130])
                    nc.sync.dma_start(ht3b[:, 0:(R + 2) * 192],
                                      ht3_d[:, r0 * 192:(r0 + R + 2) * 192])
                else:
                    hp2b, ht3b = tiles

                m = mpool.tile([128, 2048], BF16, tag="m")
                eT2 = spool.tile([128, 16 * 72], BF16, tag="eT2")

                def conv1(ic):
                    ps1 = pp1.tile([128, 512], F32)
                    for dy in range(3):
                        rhs = ap_of(hp2b, (4 * ic + dy) * 130,
                                    [[18 * 130, 128], [130, 4], [1, 128]])
                        nc.tensor.matmul(ps1[:], w1a[:, dy * 128:(dy + 1) * 128],
                                         rhs, start=(dy == 0), stop=False)
                    for dy in range(3):
                        rhs = ap_of(hp2b, (4 * ic + dy) * 130 + 2,
                                    [[18 * 130, 64], [130, 4], [1, 128]])
                        nc.tensor.matmul(ps1[:], w1b[:, dy * 128:(dy + 1) * 128],
                                         rhs, start=False, stop=(dy == 2))
                    nc.scalar.activation(m[:, ic * 512:(ic + 1) * 512], ps1[:],
                                         AF.Relu, bias=b1[:], scale=1.0)

                def conv2(ic):
                    psE = ppE.tile([128, 160], F32)
                    for rl in range(4):
                        r = 4 * ic + rl
                        nc.tensor.matmul(psE[:, rl * 40:rl * 40 + 36],
                                         m[:, r * 128:(r + 1) * 128], w2t[:])
                    # exp with free-dup x2: eT2[x, (4r, 36, 2)]
                    e_out = ap_of(eT2, ic * 4 * 72,
                                  [[16 * 72, 128], [72, 4], [2, 36], [1, 2]])
                    e_in = ap_of(psE, 0, [[160, 128], [40, 4], [1, 36], [0, 2]])
                    nc.scalar.activation(e_out, e_in, AF.Exp, scale=1.0)

                # chunk-level software pipeline: conv1(ic+1) before conv2(ic)
                nch = R // 4
                conv1(0)
                for ic in range(1, nch):
                    conv1(ic)
                    conv2(ic - 1)
                conv2(nch - 1)

                # ---- softmax pieces (transposed layout, x on partitions) ----
                q2 = spool.tile([128, 16 * 72], BF16, tag="q2")
                in_e = ap_of(eT2, 0, [[16 * 72, 128], [72, R], [1, 72]])
                in_b = ap_of(eb2, 0, [[72, 128], [0, R], [1, 72]])
                q_out = ap_of(q2, 0, [[16 * 72, 128], [72, R], [1, 72]])
                nc.vector.tensor_tensor(q_out, in_e, in_b, ALU.mult)

                zt = spool.tile([128, 64], F32, tag="zt")
                rz = spool.tile([128, 64], F32, tag="rz")
                rzd = spool.tile([128, 128], BF16, tag="rzd")
                z_in = ap_of(q2, 0, [[16 * 72, 128], [72, R], [18, 4], [2, 9]])
                z_out = ap_of(zt, 0, [[64, 128], [1, R * 4]])
                nc.vector.tensor_reduce(z_out, z_in, AX.X, ALU.add)
                nc.vector.reciprocal(ap_of(rz, 0, [[64, 128], [1, R * 4]]),
                                     ap_of(zt, 0, [[64, 128], [1, R * 4]]))
                rzd_out = ap_of(rzd, 0, [[128, 128], [2, R * 4], [1, 2]])
                rzd_in = ap_of(rz, 0, [[64, 128], [1, R * 4], [0, 2]])
                nc.scalar.copy(rzd_out, rzd_in)

                # nm[x, (ff, r, t, 2)] = q2 * rz  (bf16, dup x2 for 2x mode)
                nm = spool.tile([128, NFF * 16 * 18], BF16, tag="nm")
                for ff in range(NFF):
                    o = ap_of(nm, ff * R * 18,
                              [[NFF * 16 * 18, 128], [18, R], [1, 18]])
                    i0 = ap_of(q2, ff * 18, [[16 * 72, 128], [72, R], [1, 18]])
                    i1 = ap_of(rzd, ff * 2, [[128, 128], [8, R], [0, 9], [1, 2]])
                    nc.vector.tensor_tensor(o, i0, i1, ALU.mult)
                return ht3b, nm

            def emit_products(R, ht3b, nm):
                """DVE tap products + PE identity-matmul tap-sum -> acc."""
                n = R * 64
                acc = accp.tile([128, NFF * 1024], BF16, tag="acc")

                def prod_ap(prod, t):
                    return ap_of(prod, t * n,
                                 [[NT * 1024, 128], [64, R], [2, 32], [1, 2]])

                def tap_ins(ff, t):
                    dy, dx = t // 3, t % 3
                    i0 = ap_of(ht3b, dy * 192 + dx * 64,
                               [[18 * 192, 128], [192, R], [2, 32], [1, 2]])
                    i1 = ap_of(nm, ff * R * 18 + t * 2,
                               [[NFF * 16 * 18, 128], [18, R], [0, 32], [1, 2]])
                    return i0, i1

                def finish_ff(ff, prod, tD):
                    # DVE: D = p6 + (p7 + p8)
                    nc.vector.tensor_add(
                        ap_of(tD, n, [[2 * 1024, 128], [1, n]]),
                        ap_of(prod, 6 * n, [[NT * 1024, 128], [1, n]]),
                        ap_of(tD, 0, [[2 * 1024, 128], [1, n]]))
                    for half in range(n // 512):
                        psacc = ppA.tile([128, 512], F32)
                        for t in range(SPLIT_K):
                            rhs = ap_of(prod, t * n + half * 512,
                                        [[NT * 1024, 128], [1, 512]])
                            nc.tensor.matmul(psacc[:], idq[:], rhs,
                                             start=(t == 0), stop=False)
                        rhs = ap_of(tD, n + half * 512, [[2 * 1024, 128], [1, 512]])
                        nc.tensor.matmul(psacc[:], idq[:], rhs,
                                         start=False, stop=True)
                        # acc layout: [x, (r, fx, fy, c)]; ff = fy*2 + fx
                        fy, fx = ff // 2, ff % 2
                        a_out = ap_of(acc, (half * 8) * 256 + fx * 128 + fy * 64,
                                      [[NFF * 1024, 128], [256, 8], [1, 64]])
                        a_in = ap_of(psacc, 0, [[512, 128], [64, 8], [1, 64]])
                        nc.scalar.copy(a_out, a_in)

                pending = None
                for ff in range(NFF):
                    prod = prodp.tile([128, NT * 1024], BF16, tag="prod")
                    tD = dpool.tile([128, 2 * 1024], BF16, tag="tD")
                    for t in range(NT):
                        i0, i1 = tap_ins(ff, t)
                        nc.vector.tensor_tensor(prod_ap(prod, t), i0, i1, ALU.mult)
                    nc.vector.tensor_add(
                        ap_of(tD, 0, [[2 * 1024, 128], [1, n]]),
                        ap_of(prod, 7 * n, [[NT * 1024, 128], [1, n]]),
                        ap_of(prod, 8 * n, [[NT * 1024, 128], [1, n]]))
                    if pending is not None:
                        finish_ff(*pending)
                    pending = (ff, prod, tD)
                finish_ff(*pending)
                return acc

            def emit_out(r0, R, acc):
                """Pixel-shuffle transposes + copies + output DMA."""
                for yg in range(R // 4):
                    psoB = ppo.tile([128, 1024], BF16)
                    for yl_loc in range(4):
                        yl = yg * 4 + yl_loc
                        for fx in range(2):
                            t_in = ap_of(acc, yl * 256 + fx * 128,
                                         [[NFF * 1024, 128], [1, 128]])
                            nc.tensor.transpose(
                                psoB[:, yl_loc * 256 + fx * 128:
                                     yl_loc * 256 + (fx + 1) * 128],
                                t_in, idq[:])
                    orow4 = opool.tile([128, 1024], F32)
                    for fx in range(2):
                        o_ap = ap_of(orow4, fx, [[1024, 128], [256, 4], [2, 128]])
                        i_ap = ap_of(psoB, fx * 128,
                                     [[1024, 128], [256, 4], [1, 128]])
                        nc.scalar.copy(o_ap, i_ap)
                    for fy in range(2):
                        od = ap_of(out_d, (2 * r0 + 8 * yg + fy) * 256,
                                   [[2 * H * W, 64], [512, 4], [1, 256]])
                        nc.sync.dma_start(od, orow4[fy * 64:(fy + 1) * 64, :])

            # block-level software pipeline: defer each block's output until
            # after the next block's conv/softmax is emitted, so the PE queue
            # is never blocked on DVE products when a new conv could run.
            prev_out = None
            first_tiles = (hp2b0, ht3b0)
            for bi, (r0, R) in enumerate(BLOCKS):
                ht3b, nm = emit_front(r0, R, first_tiles if bi == 0 else None)
                if prev_out is not None:
                    emit_out(*prev_out)
                acc = emit_products(R, ht3b, nm)
                prev_out = (r0, R, acc)
            emit_out(*prev_out)

    nc.compile()
    return nc


def prep_shared(W1, b1, W2, b2):
    W1 = np.asarray(W1, np.float32)
    b1 = np.asarray(b1, np.float32)
    W2 = np.asarray(W2, np.float32).reshape(36, 128)
    b2 = np.asarray(b2, np.float32)

    w1a = np.zeros((128, 3 * 128), np.float32)
    w1b = np.zeros((64, 3 * 128), np.float32)
    for dy in range(3):
        w1a[0:64, dy * 128:(dy + 1) * 128] = W1[:, :, dy, 0].T
        w1a[64:128, dy * 128:(dy + 1) * 128] = W1[:, :, dy, 1].T
        w1b[:, dy * 128:(dy + 1) * 128] = W1[:, :, dy, 2].T

    # w2t columns k = ff*9 + t  ->  original channel t*4 + ff, 0.25 folded in
    o_of_mp = np.array([t * 4 + ff for ff in range(4) for t in range(9)])
    w2t = np.ascontiguousarray((0.25 * W2[o_of_mp, :]).T)
    eb2 = np.exp(0.25 * b2[o_of_mp]).astype(np.float32)        # [36]
    eb2d = np.broadcast_to(np.repeat(eb2, 2)[None, :], (128, 72))

    bf = ml_dtypes.bfloat16
    return {
        "w1a": w1a.astype(bf), "w1b": w1b.astype(bf),
        "b1c": b1.reshape(128, 1).astype(np.float32),
        "w2t": w2t.astype(bf),
        "eb2d": np.ascontiguousarray(eb2d).astype(bf),
        "idq": np.eye(128, dtype=bf),
    }


def kernel(h, W1, b1, W2, b2, _trace=False):
    h = np.asarray(h, np.float32)
    shared = prep_shared(W1, b1, W2, b2)

    hp = np.pad(h, ((0, 0), (0, 0), (1, 1), (1, 1)))  # [B, C, 130, 130]
    bf = ml_dtypes.bfloat16
    in_maps = []
    for core in range(NCORES):
        b, half = core // 2, core % 2
        y0 = half * HL
        win = hp[b, :, y0:y0 + 66, :]  # [64, 66, 130]
        hp2 = np.zeros((128, 66, 130), np.float32)
        hp2[0:64] = win
        hp2[64:128, :, 0:129] = win[:, :, 1:130]
        # ht3[x, y, dx, c] = 8*win[c, y, x+dx]
        w8 = 8.0 * win
        ht3 = np.stack([w8[:, :, dx:dx + 128] for dx in range(3)],
                       axis=0).transpose(3, 2, 0, 1)  # [128, 66, 3, 64]
        m = dict(shared)
        m["hp2"] = np.ascontiguousarray(hp2.reshape(128, -1)).astype(bf)
        m["ht3"] = np.ascontiguousarray(ht3.reshape(128, -1)).astype(bf)
        in_maps.append(m)

    if "nc" not in _cached:
        _cached["nc"] = build_nc()
    res = run_bass_kernel_spmd(_cached["nc"], in_maps, core_ids=list(range(NCORES)),
                               trace=_trace)

    out = np.zeros((B, C, 2 * H, 2 * W), np.float32)
    for core in range(NCORES):
        b, half = core // 2, core % 2
        out[b, :, half * 128:(half + 1) * 128, :] = \
            res.results[core]["out"].reshape(64, H, 2 * W)
    if _trace:
        return out, res
    return out
